# revision 32
# baseline (speedup 1.0000x reference)
"""Trainium2 Bass kernel for DMTetGeometry marching-tets (R=64 Kuhn grid).

Strategy: the tet mesh from the problem's setup is a regular Kuhn-split
grid, so edge "uniqueness" is analytic: every sorted tet edge is
(a, a+delta) with delta in 7 classes, and jnp.unique's lexicographic
order == (vertex a ascending, delta ascending).  Marching tets then
becomes a regular stencil + prefix-sum + compaction problem:

  - 8 NeuronCores, sharded by x-slabs of the vertex lattice (the tets
    and their derived edges shard along x with a 1-plane halo).
  - each core: occupancy classification, per-class crossing masks M,
    edge-interpolation products, and per-tet config indices, all as
    dense [128, free] vector ops (device).
  - host: input slab/halo layout prep, the global exclusive scan of M
    (edge rank <-> dedup), compaction, and triangle-table assembly.
"""
import sys
import os

sys.path.insert(0, '/opt/trn_rl_repo')

import numpy as np

R = 64
NV = R + 1  # 65

TRIANGLE_TABLE = np.array([
    [-1, -1, -1, -1, -1, -1], [1, 0, 2, -1, -1, -1], [4, 0, 3, -1, -1, -1], [1, 4, 2, 1, 3, 4],
    [3, 1, 5, -1, -1, -1], [2, 3, 0, 2, 5, 3], [1, 4, 0, 1, 5, 4], [4, 2, 5, -1, -1, -1],
    [4, 5, 2, -1, -1, -1], [4, 1, 0, 4, 5, 1], [3, 2, 0, 3, 5, 2], [1, 3, 5, -1, -1, -1],
    [4, 1, 2, 4, 3, 1], [3, 0, 4, -1, -1, -1], [2, 0, 1, -1, -1, -1], [-1, -1, -1, -1, -1, -1]],
    dtype=np.int32)
BASE_TET_EDGES = np.array([0, 1, 0, 2, 0, 3, 1, 2, 1, 3, 2, 3], dtype=np.int32).reshape(6, 2)
KUHN = np.array([[0, 1, 3, 7], [0, 2, 3, 7], [0, 1, 5, 7], [0, 2, 6, 7], [0, 4, 5, 7], [0, 4, 6, 7]])
OFFS = np.array([(0, 0, 0), (1, 0, 0), (0, 1, 0), (1, 1, 0),
                 (0, 0, 1), (1, 0, 1), (0, 1, 1), (1, 1, 1)])
# edge classes sorted by delta = (dx*65+dy)*65+dz
CLASS_D = np.array([(0, 0, 1), (0, 1, 0), (0, 1, 1), (1, 0, 0),
                    (1, 0, 1), (1, 1, 0), (1, 1, 1)])

# device layout constants
NT = 5            # row tiles per core
PR = [128, 128, 128, 128, 82]  # live partitions per tile (rows < 594)
PROWS = 128
ROWS = NT * PROWS          # 640 rows; row = xl*66 + y (y has a guard row at 65)
SG_ROWS = 728              # guarded sdf rows incl. padding
W1 = 4 * 66                # sdfblob cols: A|B|C|D z-columns (z guard at 65)
WM = NV * 7                # 455: interleaved (z, class) block width
WB = 456                   # even-padded block width (scratch tiles)
WVA = 3 * 66               # vd A-column coords x|y|z
W2 = 4 * WM                # intl cols: s1g|vbx|vby|vbz
WTI = R * 6                # 384: tet config cols (k-major: k*64+z)
WO1 = 4 * WM               # fp32 out: Px|Py|Pz|M
WO2 = WTI                  # out2: TI_raw (sign-sum)

_NC_CACHE = {}


def _build_bass():
    if 'nc' in _NC_CACHE:
        return _NC_CACHE['nc']
    import concourse.bass as bass
    import concourse.mybir as mybir
    from concourse.alu_op_type import AluOpType
    import bass_rust
    from contextlib import ExitStack

    nc = bass.Bass()
    f32 = mybir.dt.float32
    sdfblob = nc.declare_dram_parameter("sdfblob", [ROWS, W1 + WVA], f32, isOutput=False)
    intl = nc.declare_dram_parameter("intl", [ROWS, W2], f32, isOutput=False)
    out1 = nc.declare_dram_parameter("out1", [ROWS, WO1], f32, isOutput=True)
    out2 = nc.declare_dram_parameter("out2", [ROWS, WO2], mybir.dt.bfloat16, isOutput=True)

    es = ExitStack()
    sb1 = [es.enter_context(nc.sbuf_tensor(f'sb1_{t}', [PROWS, W1 + WVA], f32)) for t in range(NT)]
    sb2 = [es.enter_context(nc.sbuf_tensor(f'sb2_{t}', [PROWS, W2], f32)) for t in range(NT)]
    ob1 = [es.enter_context(nc.sbuf_tensor(f'ob1_{t}', [PROWS, WO1], f32)) for t in range(NT)]
    ob2 = [es.enter_context(nc.sbuf_tensor(f'ob2_{t}', [PROWS, WO2], mybir.dt.bfloat16)) for t in range(NT)]
    sg4 = [es.enter_context(nc.sbuf_tensor(f'sg4_{t}', [PROWS, W1], f32)) for t in range(NT)]
    sg1 = [es.enter_context(nc.sbuf_tensor(f'sg1_{t}', [PROWS, WB], f32)) for t in range(NT)]
    tmpd = es.enter_context(nc.sbuf_tensor('tmpd', [PROWS, WB], f32))
    tmpp = es.enter_context(nc.sbuf_tensor('tmpp', [PROWS, WB], f32))
    base = es.enter_context(nc.sbuf_tensor('base', [PROWS, R], f32))
    tk = es.enter_context(nc.sbuf_tensor('tk', [PROWS, R], f32))
    tk2 = es.enter_context(nc.sbuf_tensor('tk2', [PROWS, 2 * R], f32))

    dsems = [es.enter_context(nc.semaphore(f"dsem{t}")) for t in range(NT)]
    d2sems = [es.enter_context(nc.semaphore(f"d2sem{t}")) for t in range(NT)]
    asem = es.enter_context(nc.semaphore("asem"))
    vsem = es.enter_context(nc.semaphore("vsem"))
    psem = es.enter_context(nc.semaphore("psem"))
    osem = es.enter_context(nc.semaphore("osem"))
    blk = es.enter_context(nc.Block())

    kinfo = []
    for k in range(6):
        cs = OFFS[KUHN[k]]
        col = [int((2 * c[0] + c[1]) * 66 + c[2]) for c in cs]
        kinfo.append((col[1], col[2]))

    def bcast7(ap_src):
        # [128, 65] slice -> [128, 65, 7] with step-0 class broadcast
        return bass_rust.AP(tensor=ap_src.tensor, offset=ap_src.offset,
                            ap=[ap_src.ap[0], [1, NV], [0, 7]])

    def zc(ap_flat):
        # [128, 455] flat block -> [128, 65, 7]
        return ap_flat.rearrange("p (z c) -> p z c", z=NV, c=7)

    @blk.sync
    def _(sync):
        for t in range(NT):
            n = PR[t]
            sl = slice(t * PROWS, t * PROWS + n)
            sync.dma_start(out=sb1[t][0:n, :], in_=sdfblob[sl, :]).then_inc(dsems[t], 16)
            sync.dma_start(out=sb2[t][0:n, :], in_=intl[sl, :]).then_inc(d2sems[t], 16)
        for t in range(NT):
            n = PR[t]
            sl = slice(t * PROWS, t * PROWS + n)
            sync.wait_ge(vsem, t + 1)
            sync.wait_ge(psem, t + 1)
            sync.dma_start(out=out1[sl, :], in_=ob1[t][0:n, :]).then_inc(osem, 16)
            sync.dma_start(out=out2[sl, :], in_=ob2[t][0:n, :]).then_inc(osem, 16)

    @blk.scalar
    def _(scalar):
        Sign = bass_rust.ActivationFunctionType.Sign
        for t in range(NT):
            n = PR[t]
            scalar.wait_ge(dsems[t], 16)
            scalar.activation(sg4[t][0:n, :], sb1[t][0:n, 0:W1], Sign).then_inc(asem, 1)
            scalar.wait_ge(d2sems[t], 16)
            scalar.activation(sg1[t][0:n, 0:WM], sb2[t][0:n, 0:WM], Sign).then_inc(asem, 1)

    @blk.gpsimd
    def _(pool):
        sub = AluOpType.subtract
        mult = AluOpType.mult
        for t in range(NT):
            n = PR[t]
            pool.wait_ge(dsems[t], 16)
            pool.wait_ge(d2sems[t], 16)
            n = PR[t]
            s0b = bcast7(sb1[t][0:n, 0:NV])
            s1 = sb2[t][0:n, 0:WM]
            # Py fully on pool (w=1)
            for w in (1,):
                va = bcast7(sb1[t][0:n, W1 + w * 66:W1 + w * 66 + NV])
                vb = sb2[t][0:n, (1 + w) * WM:(2 + w) * WM]
                dst = ob1[t][0:n, w * WM:(1 + w) * WM]
                pool.tensor_tensor(zc(dst), zc(vb), s0b, mult)
                pool.tensor_tensor(zc(tmpp[0:n, 0:WM]), va, zc(s1), mult)
                pool.tensor_tensor(zc(dst), zc(dst), zc(tmpp[0:n, 0:WM]), sub)
            # M = sign(s0) != sign(s1) on pool (fp32 out)
            pool.wait_ge(asem, 2 * t + 2)
            sg0b = bcast7(sg4[t][0:n, 0:NV])
            inst = pool.tensor_tensor(zc(ob1[t][0:n, 3 * WM:4 * WM]), sg0b,
                                      zc(sg1[t][0:n, 0:WM]), sub)
            inst.then_inc(psem, 1)

    @blk.vector
    def _(vector):
        ne = AluOpType.not_equal
        sub = AluOpType.subtract
        mult = AluOpType.mult
        add = AluOpType.add
        for t in range(NT):
            n = PR[t]
            vector.wait_ge(asem, 2 * t + 1)
            s0b = bcast7(sb1[t][0:n, 0:NV])
            sg0b = bcast7(sg4[t][0:n, 0:NV])
            # tet configs from sign field: raw = sA + 2*c1 + 4*c2 + 8*sD1
            vector.scalar_tensor_tensor(
                base[0:n], sg4[t][0:n, 3 * 66 + 1:3 * 66 + 1 + R], 8.0, sg4[t][0:n, 0:R],
                mult, add)

            def pair_ap(src_t, col_a, col_b):
                a0 = src_t[:, 0:1]
                return bass_rust.AP(tensor=a0.tensor, offset=a0.offset + col_a,
                                    ap=[a0.ap[0], [col_b - col_a, 2], [1, R]])

            base_b = bass_rust.AP(tensor=base[0:n].tensor, offset=base[0:n].offset,
                                  ap=[base[0:n].ap[0], [0, 2], [1, R]])
            tk2_v = tk2[0:n].rearrange("p (j z) -> p j z", j=2, z=R)
            for pidx in range(3):
                ka, kb = 2 * pidx, 2 * pidx + 1
                c1a, c2a = kinfo[ka]
                c1b, c2b = kinfo[kb]
                vector.scalar_tensor_tensor(
                    tk2_v, pair_ap(sg4[t][0:n], c1a, c1b), 2.0, base_b, mult, add)
                vector.scalar_tensor_tensor(
                    ob2[t][0:n, ka * R:(kb + 1) * R].rearrange("p (j z) -> p j z", j=2, z=R),
                    pair_ap(sg4[t][0:n], c2a, c2b), 4.0, tk2_v, mult, add)
            # Px, Pz on DVE (w = 0, 2)
            vector.wait_ge(d2sems[t], 16)
            s1 = sb2[t][0:n, 0:WM]
            for w in (0, 2):
                va = bcast7(sb1[t][0:n, W1 + w * 66:W1 + w * 66 + NV])
                vb = sb2[t][0:n, (1 + w) * WM:(2 + w) * WM]
                dst = ob1[t][0:n, w * WM:(1 + w) * WM]
                vector.tensor_tensor(zc(dst), zc(vb), s0b, mult)
                vector.tensor_tensor(zc(tmpd[0:n, 0:WM]), va, zc(s1), mult)
                inst = vector.tensor_tensor(zc(dst), zc(dst), zc(tmpd[0:n, 0:WM]), sub)
            inst.then_inc(vsem, 1)

    _NC_CACHE['nc'] = (nc, es)
    return _NC_CACHE['nc']


def _host_prep(sdf3, vd3):
    """Per-core input arrays. Returns list of dicts."""
    in_maps = []
    for i in range(8):
        x0 = 8 * i
        xs = np.minimum(np.arange(x0, x0 + 10), NV - 1)
        # guarded sdf slab [10, 66, 66]
        G = np.empty((10, 66, 66), np.float32)
        G[:, :65, :65] = sdf3[xs]
        G[:, 65, :65] = G[:, 64, :65]
        G[:, :, 65] = G[:, :, 64]
        sgr = np.zeros((SG_ROWS, 66), np.float32)
        sgr[:660] = G.reshape(660, 66)
        rows = np.arange(ROWS)
        sdfblob = np.concatenate(
            [sgr[rows], sgr[rows + 1], sgr[rows + 66], sgr[rows + 67]], axis=1)

        # guarded vd slab [10, 66, 66, 3]
        V = np.empty((10, 66, 66, 3), np.float32)
        V[:, :65, :65] = vd3[xs]
        V[:, 65, :65] = V[:, 64, :65]
        V[:, :, 65] = V[:, :, 64]
        vgr = np.zeros((SG_ROWS, 66, 3), np.float32)
        vgr[:660] = V.reshape(660, 66, 3)

        # interleaved candidate streams [640, 455]:
        # cand (row, z, c): a = lattice(row), b = a + delta_c
        dz = CLASS_D[:, 2]
        drow = CLASS_D[:, 0] * 66 + CLASS_D[:, 1]
        z_idx = np.arange(NV)
        # source row/col for b-side, per (row, z, c)
        brow = rows[:, None, None] + drow[None, None, :]        # [640,1,7]
        bz = z_idx[None, :, None] + dz[None, None, :]           # [1,65,7]
        s0rep = np.repeat(sgr[rows][:, :65, None], 7, axis=2).reshape(ROWS, WM)
        s1g = sgr[brow, bz].reshape(ROWS, WM)
        # edge-class presence: every advanced axis must be < R (the guard
        # dup only zeroes the diff when ALL advanced axes hit the guard,
        # so diagonal classes at a boundary need explicit zeroing).
        xl = rows // 66
        yl = rows % 66
        xg = np.minimum(x0 + xl, NV)
        pres = ((CLASS_D[None, None, :, 0] == 0) | (xg[:, None, None] < R)) & \
               ((CLASS_D[None, None, :, 1] == 0) | (yl[:, None, None] < R)) & \
               ((CLASS_D[None, None, :, 2] == 0) | (z_idx[None, :, None] < R))
        pres = pres.reshape(ROWS, WM)
        s1g = np.where(pres, s1g, s0rep)
        vb = [vgr[brow, bz, w].reshape(ROWS, WM) for w in range(3)]
        intl = np.concatenate([s1g] + vb, axis=1).astype(np.float32)
        vda = np.ascontiguousarray(vgr[rows].transpose(0, 2, 1).reshape(ROWS, 3 * 66)).astype(np.float32)
        in_maps.append({"sdfblob": np.ascontiguousarray(np.concatenate([sdfblob, vda], axis=1)),
                        "intl": np.ascontiguousarray(intl)})
    return in_maps


def _host_post(results, sdf3):
    """Assemble global outputs from per-core device results."""
    M3 = np.zeros((NV, NV, NV, 7), dtype=bool)
    P3 = np.zeros((NV, NV, NV, 7, 3), np.float32)
    TI3 = np.zeros((R, R, R, 6), np.int32)
    for i in range(8):
        o1 = results[i]["out1"]
        o2 = np.asarray(results[i]["out2"], dtype=np.float32)
        x0 = 8 * i
        npl = 8 if i < 7 else 9
        rows = (np.arange(npl)[:, None] * 66 + np.arange(65)[None, :]).reshape(-1)
        M3[x0:x0 + npl] = o1[rows, 3 * WM:4 * WM].reshape(npl, 65, NV, 7) != 0.0
        for w in range(3):
            P3[x0:x0 + npl, :, :, :, w] = o1[rows, w * WM:(1 + w) * WM].reshape(npl, 65, NV, 7)
        ncx = 8
        crows = (np.arange(ncx)[:, None] * 66 + np.arange(64)[None, :]).reshape(-1)
        ti_raw = np.rint(o2[crows, 0:WTI]).astype(np.int32).reshape(ncx, 64, 6, R)
        TI3[x0:x0 + ncx] = ((ti_raw + 15) // 2).transpose(0, 1, 3, 2)

    # global exclusive scan in (x, y, z, c) order
    Mf = M3.reshape(-1)
    Rk = (np.cumsum(Mf) - Mf).astype(np.int64).reshape(NV, NV, NV, 7)

    # verts: interp values at crossing edges, rank order; denominators
    # recomputed host-side only at the ~2% selected candidates
    ax, ay, az, ac = np.nonzero(M3)
    d = CLASS_D[ac]
    s0 = sdf3[ax, ay, az]
    s1 = sdf3[ax + d[:, 0], ay + d[:, 1], az + d[:, 2]]
    sel = np.nonzero(Mf)[0]
    Pf = P3.reshape(-1, 3)
    out_verts = (Pf[sel] / (s0 - s1)[:, None]).astype(np.float32)

    # faces
    ti_flat = TI3.reshape(-1)
    E6 = np.zeros((R, R, R, 6, 6), np.int64)
    for k in range(6):
        vs = OFFS[KUHN[k]]
        for e in range(6):
            i_, j_ = BASE_TET_EDGES[e]
            a, b = vs[i_], vs[j_]
            d = b - a
            c = int(np.nonzero((CLASS_D == d).all(1))[0][0])
            dx, dy, dzz = a
            E6[:, :, :, k, e] = Rk[dx:dx + R, dy:dy + R, dzz:dzz + R, c]
    E6f = E6.reshape(-1, 6)
    s = np.array([0, 1, 1, 2, 1, 2, 2, 3, 1, 2, 2, 3, 2, 3, 3, 4])[ti_flat]
    num_tri = np.minimum(s, 4 - s)
    m1 = num_tri == 1
    m2 = num_tri == 2
    tt = TRIANGLE_TABLE[ti_flat]
    f1 = np.take_along_axis(E6f[m1], tt[m1][:, :3].astype(np.int64), axis=1)
    f2 = np.take_along_axis(E6f[m2], tt[m2][:, :6].astype(np.int64), axis=1).reshape(-1, 3)
    faces = np.concatenate([f1, f2], 0).astype(np.int32)

    num_tets = R * R * R * 6
    tet_gidx = np.arange(num_tets)
    face_gidx = np.concatenate(
        [tet_gidx[m1] * 2,
         np.stack([tet_gidx[m2] * 2, tet_gidx[m2] * 2 + 1], -1).reshape(-1)], 0)

    N = int(np.ceil(np.sqrt((num_tets * 2 + 1) // 2)))
    lin = np.linspace(0.0, 1.0 - 1.0 / N, N, dtype=np.float32)
    tex_y, tex_x = np.meshgrid(lin, lin, indexing='ij')
    pad = np.float32(0.9 / N)
    uvs = np.stack([tex_x, tex_y, tex_x + pad, tex_y,
                    tex_x + pad, tex_y + pad, tex_x, tex_y + pad], -1).reshape(-1, 2).astype(np.float32)
    tet_idx = face_gidx // 2
    tri_idx = face_gidx % 2
    uv_idx = np.stack([tet_idx * 4, tet_idx * 4 + tri_idx + 1,
                       tet_idx * 4 + tri_idx + 2], -1).astype(np.int32)
    return out_verts, faces, uvs, uv_idx


_LAST_RESULTS = {}


def _expected_tets():
    def vid(x, y, z):
        return (x * NV + y) * NV + z
    xs, ys, zs = np.meshgrid(np.arange(R), np.arange(R), np.arange(R), indexing='ij')
    corners = np.stack([vid(xs + dx, ys + dy, zs + dz) for dx, dy, dz in OFFS], -1).reshape(-1, 8)
    return corners[:, KUHN].reshape(-1, 4)


def _general_fallback(verts, sdf, deform, indices, grid_res):
    """Reference-faithful numpy path for non-grid connectivity (not expected)."""
    vd = (verts + (2.0 / (grid_res * 2)) * np.tanh(deform)).astype(np.float32)
    occ_n = sdf > 0
    occ_fx4 = occ_n[indices.reshape(-1)].reshape(-1, 4)
    occ_sum = occ_fx4.sum(-1)
    valid = (occ_sum > 0) & (occ_sum < 4)
    vt = indices[valid]
    all_edges = np.sort(vt[:, BASE_TET_EDGES.reshape(-1)].reshape(-1, 2), axis=1)
    unique_edges, idx_map = np.unique(all_edges, axis=0, return_inverse=True)
    idx_map = idx_map.reshape(-1)
    mask_edges = occ_n[unique_edges].sum(-1) == 1
    mapping = np.where(mask_edges, np.cumsum(mask_edges) - 1, -1)
    idx_map = mapping[idx_map].reshape(-1, 6)
    iv = unique_edges[mask_edges]
    s0, s1 = sdf[iv[:, 0]], sdf[iv[:, 1]]
    d = s0 - s1
    ov = (vd[iv[:, 0]] * (-s1 / d)[:, None] + vd[iv[:, 1]] * (s0 / d)[:, None]).astype(np.float32)
    v_id = np.array([1, 2, 4, 8])
    ti = (occ_fx4[valid] * v_id).sum(-1)
    num_tri = np.array([0, 1, 1, 2, 1, 2, 2, 1, 1, 2, 2, 1, 2, 1, 1, 0])[ti]
    m1, m2 = num_tri == 1, num_tri == 2
    tt = TRIANGLE_TABLE[ti]
    f1 = np.take_along_axis(idx_map[m1], tt[m1][:, :3].astype(np.int64), axis=1)
    f2 = np.take_along_axis(idx_map[m2], tt[m2][:, :6].astype(np.int64), axis=1).reshape(-1, 3)
    faces = np.concatenate([f1, f2], 0).astype(np.int32)
    nt = indices.shape[0]
    tg = np.arange(nt)[valid]
    fg = np.concatenate([tg[m1] * 2, np.stack([tg[m2] * 2, tg[m2] * 2 + 1], -1).reshape(-1)], 0)
    N = int(np.ceil(np.sqrt((nt * 2 + 1) // 2)))
    lin = np.linspace(0.0, 1.0 - 1.0 / N, N, dtype=np.float32)
    ty, tx = np.meshgrid(lin, lin, indexing='ij')
    pad = np.float32(0.9 / N)
    uvs = np.stack([tx, ty, tx + pad, ty, tx + pad, ty + pad, tx, ty + pad], -1).reshape(-1, 2).astype(np.float32)
    tix = (fg // 2 // N) * N + (fg // 2 % N)
    tri = fg % 2
    uv_idx = np.stack([tix * 4, tix * 4 + tri + 1, tix * 4 + tri + 2], -1).astype(np.int32)
    return ov, faces, uvs, uv_idx


def kernel(verts, sdf, deform, indices, grid_res):
    from concourse.bass_utils import run_bass_kernel_spmd
    verts = np.asarray(verts, dtype=np.float32)
    sdf = np.asarray(sdf, dtype=np.float32)
    deform = np.asarray(deform, dtype=np.float32)
    indices = np.asarray(indices)
    gr = int(np.asarray(grid_res))
    if gr != R or indices.shape != (R * R * R * 6, 4) or             not np.array_equal(indices.astype(np.int64), _expected_tets()):
        return _general_fallback(verts, sdf, deform, indices.astype(np.int64), gr)

    vd = (verts + (2.0 / (gr * 2)) * np.tanh(deform, dtype=np.float32)).astype(np.float32)
    sdf3 = sdf.reshape(NV, NV, NV)
    vd3 = vd.reshape(NV, NV, NV, 3)

    nc, _ = _build_bass()
    in_maps = _host_prep(sdf3, vd3)
    res = run_bass_kernel_spmd(nc, in_maps, list(range(8)))
    _LAST_RESULTS['res'] = res
    return _host_post(res.results, sdf3)


# revision 33
# speedup vs baseline: 1.0065x; 1.0065x over previous
"""Trainium2 Bass kernel for DMTetGeometry marching-tets (R=64 Kuhn grid).

Strategy: the tet mesh from the problem's setup is a regular Kuhn-split
grid, so edge "uniqueness" is analytic: every sorted tet edge is
(a, a+delta) with delta in 7 classes, and jnp.unique's lexicographic
order == (vertex a ascending, delta ascending).  Marching tets then
becomes a regular stencil + prefix-sum + compaction problem:

  - 8 NeuronCores, sharded by x-slabs of the vertex lattice (the tets
    and their derived edges shard along x with a 1-plane halo).
  - each core: occupancy classification, per-class crossing masks M,
    edge-interpolation products, and per-tet config indices, all as
    dense [128, free] vector ops (device).
  - host: input slab/halo layout prep, the global exclusive scan of M
    (edge rank <-> dedup), compaction, and triangle-table assembly.
"""
import sys
import os

sys.path.insert(0, '/opt/trn_rl_repo')

import numpy as np

R = 64
NV = R + 1  # 65

TRIANGLE_TABLE = np.array([
    [-1, -1, -1, -1, -1, -1], [1, 0, 2, -1, -1, -1], [4, 0, 3, -1, -1, -1], [1, 4, 2, 1, 3, 4],
    [3, 1, 5, -1, -1, -1], [2, 3, 0, 2, 5, 3], [1, 4, 0, 1, 5, 4], [4, 2, 5, -1, -1, -1],
    [4, 5, 2, -1, -1, -1], [4, 1, 0, 4, 5, 1], [3, 2, 0, 3, 5, 2], [1, 3, 5, -1, -1, -1],
    [4, 1, 2, 4, 3, 1], [3, 0, 4, -1, -1, -1], [2, 0, 1, -1, -1, -1], [-1, -1, -1, -1, -1, -1]],
    dtype=np.int32)
BASE_TET_EDGES = np.array([0, 1, 0, 2, 0, 3, 1, 2, 1, 3, 2, 3], dtype=np.int32).reshape(6, 2)
KUHN = np.array([[0, 1, 3, 7], [0, 2, 3, 7], [0, 1, 5, 7], [0, 2, 6, 7], [0, 4, 5, 7], [0, 4, 6, 7]])
OFFS = np.array([(0, 0, 0), (1, 0, 0), (0, 1, 0), (1, 1, 0),
                 (0, 0, 1), (1, 0, 1), (0, 1, 1), (1, 1, 1)])
# edge classes sorted by delta = (dx*65+dy)*65+dz
CLASS_D = np.array([(0, 0, 1), (0, 1, 0), (0, 1, 1), (1, 0, 0),
                    (1, 0, 1), (1, 1, 0), (1, 1, 1)])

# device layout constants
NT = 5            # row tiles per core
PR = [128, 128, 128, 128, 82]  # live partitions per tile (rows < 594)
PROWS = 128
ROWS = NT * PROWS          # 640 rows; row = xl*66 + y (y has a guard row at 65)
SG_ROWS = 728              # guarded sdf rows incl. padding
W1 = 4 * 66                # sdfblob cols: A|B|C|D z-columns (z guard at 65)
WM = NV * 7                # 455: interleaved (z, class) block width
WB = 456                   # even-padded block width (scratch tiles)
WVA = 3 * 66               # vd A-column coords x|y|z
W2 = 4 * WM                # intl cols: s1g|vbx|vby|vbz
WTI = R * 6                # 384: tet config cols (k-major: k*64+z)
WO1 = 3 * WM               # fp32 out: Px|Py|Pz
WO2 = WTI + WM             # out2: TI_raw | M (sign-diff), bf16

_NC_CACHE = {}


def _build_bass():
    if 'nc' in _NC_CACHE:
        return _NC_CACHE['nc']
    import concourse.bass as bass
    import concourse.mybir as mybir
    from concourse.alu_op_type import AluOpType
    import bass_rust
    from contextlib import ExitStack

    nc = bass.Bass()
    f32 = mybir.dt.float32
    sdfblob = nc.declare_dram_parameter("sdfblob", [ROWS, W1 + WVA], f32, isOutput=False)
    intl = nc.declare_dram_parameter("intl", [ROWS, W2], f32, isOutput=False)
    out1 = nc.declare_dram_parameter("out1", [ROWS, WO1], f32, isOutput=True)
    out2 = nc.declare_dram_parameter("out2", [ROWS, WO2], mybir.dt.bfloat16, isOutput=True)

    es = ExitStack()
    sb1 = [es.enter_context(nc.sbuf_tensor(f'sb1_{t}', [PROWS, W1 + WVA], f32)) for t in range(NT)]
    sb2 = [es.enter_context(nc.sbuf_tensor(f'sb2_{t}', [PROWS, W2], f32)) for t in range(NT)]
    ob1 = [es.enter_context(nc.sbuf_tensor(f'ob1_{t}', [PROWS, WO1], f32)) for t in range(NT)]
    ob2 = [es.enter_context(nc.sbuf_tensor(f'ob2_{t}', [PROWS, WO2], mybir.dt.bfloat16)) for t in range(NT)]
    sg4 = [es.enter_context(nc.sbuf_tensor(f'sg4_{t}', [PROWS, W1], f32)) for t in range(NT)]
    sg1 = [es.enter_context(nc.sbuf_tensor(f'sg1_{t}', [PROWS, WB], f32)) for t in range(NT)]
    tmpd = es.enter_context(nc.sbuf_tensor('tmpd', [PROWS, WB], f32))
    tmpp = es.enter_context(nc.sbuf_tensor('tmpp', [PROWS, WB], f32))
    base = es.enter_context(nc.sbuf_tensor('base', [PROWS, R], f32))
    tk = es.enter_context(nc.sbuf_tensor('tk', [PROWS, R], f32))
    tk2 = es.enter_context(nc.sbuf_tensor('tk2', [PROWS, 2 * R], f32))

    dsems = [es.enter_context(nc.semaphore(f"dsem{t}")) for t in range(NT)]
    d2sems = [es.enter_context(nc.semaphore(f"d2sem{t}")) for t in range(NT)]
    asem = es.enter_context(nc.semaphore("asem"))
    vsem = es.enter_context(nc.semaphore("vsem"))
    psem = es.enter_context(nc.semaphore("psem"))
    osem = es.enter_context(nc.semaphore("osem"))
    blk = es.enter_context(nc.Block())

    kinfo = []
    for k in range(6):
        cs = OFFS[KUHN[k]]
        col = [int((2 * c[0] + c[1]) * 66 + c[2]) for c in cs]
        kinfo.append((col[1], col[2]))

    def bcast7(ap_src):
        # [128, 65] slice -> [128, 65, 7] with step-0 class broadcast
        return bass_rust.AP(tensor=ap_src.tensor, offset=ap_src.offset,
                            ap=[ap_src.ap[0], [1, NV], [0, 7]])

    def zc(ap_flat):
        # [128, 455] flat block -> [128, 65, 7]
        return ap_flat.rearrange("p (z c) -> p z c", z=NV, c=7)

    @blk.sync
    def _(sync):
        for t in range(NT):
            n = PR[t]
            sl = slice(t * PROWS, t * PROWS + n)
            sync.dma_start(out=sb1[t][0:n, :], in_=sdfblob[sl, :]).then_inc(dsems[t], 16)
            sync.dma_start(out=sb2[t][0:n, :], in_=intl[sl, :]).then_inc(d2sems[t], 16)
        for t in range(NT):
            n = PR[t]
            sl = slice(t * PROWS, t * PROWS + n)
            sync.wait_ge(vsem, t + 1)
            sync.wait_ge(psem, t + 1)
            sync.dma_start(out=out1[sl, :], in_=ob1[t][0:n, :]).then_inc(osem, 16)
            sync.dma_start(out=out2[sl, :], in_=ob2[t][0:n, :]).then_inc(osem, 16)

    @blk.scalar
    def _(scalar):
        Sign = bass_rust.ActivationFunctionType.Sign
        for t in range(NT):
            n = PR[t]
            scalar.wait_ge(dsems[t], 16)
            scalar.activation(sg4[t][0:n, :], sb1[t][0:n, 0:W1], Sign).then_inc(asem, 1)
            scalar.wait_ge(d2sems[t], 16)
            scalar.activation(sg1[t][0:n, 0:WM], sb2[t][0:n, 0:WM], Sign).then_inc(asem, 1)

    @blk.gpsimd
    def _(pool):
        sub = AluOpType.subtract
        mult = AluOpType.mult
        for t in range(NT):
            n = PR[t]
            pool.wait_ge(dsems[t], 16)
            pool.wait_ge(d2sems[t], 16)
            n = PR[t]
            s0b = bcast7(sb1[t][0:n, 0:NV])
            s1 = sb2[t][0:n, 0:WM]
            # Py fully on pool (w=1)
            for w in (1,):
                va = bcast7(sb1[t][0:n, W1 + w * 66:W1 + w * 66 + NV])
                vb = sb2[t][0:n, (1 + w) * WM:(2 + w) * WM]
                dst = ob1[t][0:n, w * WM:(1 + w) * WM]
                pool.tensor_tensor(zc(dst), zc(vb), s0b, mult)
                pool.tensor_tensor(zc(tmpp[0:n, 0:WM]), va, zc(s1), mult)
                pool.tensor_tensor(zc(dst), zc(dst), zc(tmpp[0:n, 0:WM]), sub)
            # M = sign(s0) != sign(s1) on pool (fp32 out)
            pool.wait_ge(asem, 2 * t + 2)
            sg0b = bcast7(sg4[t][0:n, 0:NV])
            inst = pool.tensor_tensor(zc(ob2[t][0:n, WTI:WTI + WM]), sg0b,
                                      zc(sg1[t][0:n, 0:WM]), sub)
            inst.then_inc(psem, 1)

    @blk.vector
    def _(vector):
        ne = AluOpType.not_equal
        sub = AluOpType.subtract
        mult = AluOpType.mult
        add = AluOpType.add
        for t in range(NT):
            n = PR[t]
            vector.wait_ge(asem, 2 * t + 1)
            s0b = bcast7(sb1[t][0:n, 0:NV])
            sg0b = bcast7(sg4[t][0:n, 0:NV])
            # tet configs from sign field: raw = sA + 2*c1 + 4*c2 + 8*sD1
            vector.scalar_tensor_tensor(
                base[0:n], sg4[t][0:n, 3 * 66 + 1:3 * 66 + 1 + R], 8.0, sg4[t][0:n, 0:R],
                mult, add)

            def pair_ap(src_t, col_a, col_b):
                a0 = src_t[:, 0:1]
                return bass_rust.AP(tensor=a0.tensor, offset=a0.offset + col_a,
                                    ap=[a0.ap[0], [col_b - col_a, 2], [1, R]])

            base_b = bass_rust.AP(tensor=base[0:n].tensor, offset=base[0:n].offset,
                                  ap=[base[0:n].ap[0], [0, 2], [1, R]])
            tk2_v = tk2[0:n].rearrange("p (j z) -> p j z", j=2, z=R)
            for pidx in range(3):
                ka, kb = 2 * pidx, 2 * pidx + 1
                c1a, c2a = kinfo[ka]
                c1b, c2b = kinfo[kb]
                vector.scalar_tensor_tensor(
                    tk2_v, pair_ap(sg4[t][0:n], c1a, c1b), 2.0, base_b, mult, add)
                vector.scalar_tensor_tensor(
                    ob2[t][0:n, ka * R:(kb + 1) * R].rearrange("p (j z) -> p j z", j=2, z=R),
                    pair_ap(sg4[t][0:n], c2a, c2b), 4.0, tk2_v, mult, add)
            # Px, Pz on DVE (w = 0, 2)
            vector.wait_ge(d2sems[t], 16)
            s1 = sb2[t][0:n, 0:WM]
            for w in (0, 2):
                va = bcast7(sb1[t][0:n, W1 + w * 66:W1 + w * 66 + NV])
                vb = sb2[t][0:n, (1 + w) * WM:(2 + w) * WM]
                dst = ob1[t][0:n, w * WM:(1 + w) * WM]
                vector.tensor_tensor(zc(dst), zc(vb), s0b, mult)
                vector.tensor_tensor(zc(tmpd[0:n, 0:WM]), va, zc(s1), mult)
                inst = vector.tensor_tensor(zc(dst), zc(dst), zc(tmpd[0:n, 0:WM]), sub)
            inst.then_inc(vsem, 1)

    _NC_CACHE['nc'] = (nc, es)
    return _NC_CACHE['nc']


def _host_prep(sdf3, vd3):
    """Per-core input arrays. Returns list of dicts."""
    in_maps = []
    for i in range(8):
        x0 = 8 * i
        xs = np.minimum(np.arange(x0, x0 + 10), NV - 1)
        # guarded sdf slab [10, 66, 66]
        G = np.empty((10, 66, 66), np.float32)
        G[:, :65, :65] = sdf3[xs]
        G[:, 65, :65] = G[:, 64, :65]
        G[:, :, 65] = G[:, :, 64]
        sgr = np.zeros((SG_ROWS, 66), np.float32)
        sgr[:660] = G.reshape(660, 66)
        rows = np.arange(ROWS)
        sdfblob = np.concatenate(
            [sgr[rows], sgr[rows + 1], sgr[rows + 66], sgr[rows + 67]], axis=1)

        # guarded vd slab [10, 66, 66, 3]
        V = np.empty((10, 66, 66, 3), np.float32)
        V[:, :65, :65] = vd3[xs]
        V[:, 65, :65] = V[:, 64, :65]
        V[:, :, 65] = V[:, :, 64]
        vgr = np.zeros((SG_ROWS, 66, 3), np.float32)
        vgr[:660] = V.reshape(660, 66, 3)

        # interleaved candidate streams [640, 455]:
        # cand (row, z, c): a = lattice(row), b = a + delta_c
        dz = CLASS_D[:, 2]
        drow = CLASS_D[:, 0] * 66 + CLASS_D[:, 1]
        z_idx = np.arange(NV)
        # source row/col for b-side, per (row, z, c)
        brow = rows[:, None, None] + drow[None, None, :]        # [640,1,7]
        bz = z_idx[None, :, None] + dz[None, None, :]           # [1,65,7]
        s0rep = np.repeat(sgr[rows][:, :65, None], 7, axis=2).reshape(ROWS, WM)
        s1g = sgr[brow, bz].reshape(ROWS, WM)
        # edge-class presence: every advanced axis must be < R (the guard
        # dup only zeroes the diff when ALL advanced axes hit the guard,
        # so diagonal classes at a boundary need explicit zeroing).
        xl = rows // 66
        yl = rows % 66
        xg = np.minimum(x0 + xl, NV)
        pres = ((CLASS_D[None, None, :, 0] == 0) | (xg[:, None, None] < R)) & \
               ((CLASS_D[None, None, :, 1] == 0) | (yl[:, None, None] < R)) & \
               ((CLASS_D[None, None, :, 2] == 0) | (z_idx[None, :, None] < R))
        pres = pres.reshape(ROWS, WM)
        s1g = np.where(pres, s1g, s0rep)
        vb = [vgr[brow, bz, w].reshape(ROWS, WM) for w in range(3)]
        intl = np.concatenate([s1g] + vb, axis=1).astype(np.float32)
        vda = np.ascontiguousarray(vgr[rows].transpose(0, 2, 1).reshape(ROWS, 3 * 66)).astype(np.float32)
        in_maps.append({"sdfblob": np.ascontiguousarray(np.concatenate([sdfblob, vda], axis=1)),
                        "intl": np.ascontiguousarray(intl)})
    return in_maps


def _host_post(results, sdf3):
    """Assemble global outputs from per-core device results."""
    M3 = np.zeros((NV, NV, NV, 7), dtype=bool)
    P3 = np.zeros((NV, NV, NV, 7, 3), np.float32)
    TI3 = np.zeros((R, R, R, 6), np.int32)
    for i in range(8):
        o1 = results[i]["out1"]
        o2 = np.asarray(results[i]["out2"], dtype=np.float32)
        x0 = 8 * i
        npl = 8 if i < 7 else 9
        rows = (np.arange(npl)[:, None] * 66 + np.arange(65)[None, :]).reshape(-1)
        M3[x0:x0 + npl] = o2[rows, WTI:WTI + WM].reshape(npl, 65, NV, 7) != 0.0
        for w in range(3):
            P3[x0:x0 + npl, :, :, :, w] = o1[rows, w * WM:(1 + w) * WM].reshape(npl, 65, NV, 7)
        ncx = 8
        crows = (np.arange(ncx)[:, None] * 66 + np.arange(64)[None, :]).reshape(-1)
        ti_raw = np.rint(o2[crows, 0:WTI]).astype(np.int32).reshape(ncx, 64, 6, R)
        TI3[x0:x0 + ncx] = ((ti_raw + 15) // 2).transpose(0, 1, 3, 2)

    # global exclusive scan in (x, y, z, c) order
    Mf = M3.reshape(-1)
    Rk = (np.cumsum(Mf) - Mf).astype(np.int64).reshape(NV, NV, NV, 7)

    # verts: interp values at crossing edges, rank order; denominators
    # recomputed host-side only at the ~2% selected candidates
    ax, ay, az, ac = np.nonzero(M3)
    d = CLASS_D[ac]
    s0 = sdf3[ax, ay, az]
    s1 = sdf3[ax + d[:, 0], ay + d[:, 1], az + d[:, 2]]
    sel = np.nonzero(Mf)[0]
    Pf = P3.reshape(-1, 3)
    out_verts = (Pf[sel] / (s0 - s1)[:, None]).astype(np.float32)

    # faces
    ti_flat = TI3.reshape(-1)
    E6 = np.zeros((R, R, R, 6, 6), np.int64)
    for k in range(6):
        vs = OFFS[KUHN[k]]
        for e in range(6):
            i_, j_ = BASE_TET_EDGES[e]
            a, b = vs[i_], vs[j_]
            d = b - a
            c = int(np.nonzero((CLASS_D == d).all(1))[0][0])
            dx, dy, dzz = a
            E6[:, :, :, k, e] = Rk[dx:dx + R, dy:dy + R, dzz:dzz + R, c]
    E6f = E6.reshape(-1, 6)
    s = np.array([0, 1, 1, 2, 1, 2, 2, 3, 1, 2, 2, 3, 2, 3, 3, 4])[ti_flat]
    num_tri = np.minimum(s, 4 - s)
    m1 = num_tri == 1
    m2 = num_tri == 2
    tt = TRIANGLE_TABLE[ti_flat]
    f1 = np.take_along_axis(E6f[m1], tt[m1][:, :3].astype(np.int64), axis=1)
    f2 = np.take_along_axis(E6f[m2], tt[m2][:, :6].astype(np.int64), axis=1).reshape(-1, 3)
    faces = np.concatenate([f1, f2], 0).astype(np.int32)

    num_tets = R * R * R * 6
    tet_gidx = np.arange(num_tets)
    face_gidx = np.concatenate(
        [tet_gidx[m1] * 2,
         np.stack([tet_gidx[m2] * 2, tet_gidx[m2] * 2 + 1], -1).reshape(-1)], 0)

    N = int(np.ceil(np.sqrt((num_tets * 2 + 1) // 2)))
    lin = np.linspace(0.0, 1.0 - 1.0 / N, N, dtype=np.float32)
    tex_y, tex_x = np.meshgrid(lin, lin, indexing='ij')
    pad = np.float32(0.9 / N)
    uvs = np.stack([tex_x, tex_y, tex_x + pad, tex_y,
                    tex_x + pad, tex_y + pad, tex_x, tex_y + pad], -1).reshape(-1, 2).astype(np.float32)
    tet_idx = face_gidx // 2
    tri_idx = face_gidx % 2
    uv_idx = np.stack([tet_idx * 4, tet_idx * 4 + tri_idx + 1,
                       tet_idx * 4 + tri_idx + 2], -1).astype(np.int32)
    return out_verts, faces, uvs, uv_idx


_LAST_RESULTS = {}


def _expected_tets():
    def vid(x, y, z):
        return (x * NV + y) * NV + z
    xs, ys, zs = np.meshgrid(np.arange(R), np.arange(R), np.arange(R), indexing='ij')
    corners = np.stack([vid(xs + dx, ys + dy, zs + dz) for dx, dy, dz in OFFS], -1).reshape(-1, 8)
    return corners[:, KUHN].reshape(-1, 4)


def _general_fallback(verts, sdf, deform, indices, grid_res):
    """Reference-faithful numpy path for non-grid connectivity (not expected)."""
    vd = (verts + (2.0 / (grid_res * 2)) * np.tanh(deform)).astype(np.float32)
    occ_n = sdf > 0
    occ_fx4 = occ_n[indices.reshape(-1)].reshape(-1, 4)
    occ_sum = occ_fx4.sum(-1)
    valid = (occ_sum > 0) & (occ_sum < 4)
    vt = indices[valid]
    all_edges = np.sort(vt[:, BASE_TET_EDGES.reshape(-1)].reshape(-1, 2), axis=1)
    unique_edges, idx_map = np.unique(all_edges, axis=0, return_inverse=True)
    idx_map = idx_map.reshape(-1)
    mask_edges = occ_n[unique_edges].sum(-1) == 1
    mapping = np.where(mask_edges, np.cumsum(mask_edges) - 1, -1)
    idx_map = mapping[idx_map].reshape(-1, 6)
    iv = unique_edges[mask_edges]
    s0, s1 = sdf[iv[:, 0]], sdf[iv[:, 1]]
    d = s0 - s1
    ov = (vd[iv[:, 0]] * (-s1 / d)[:, None] + vd[iv[:, 1]] * (s0 / d)[:, None]).astype(np.float32)
    v_id = np.array([1, 2, 4, 8])
    ti = (occ_fx4[valid] * v_id).sum(-1)
    num_tri = np.array([0, 1, 1, 2, 1, 2, 2, 1, 1, 2, 2, 1, 2, 1, 1, 0])[ti]
    m1, m2 = num_tri == 1, num_tri == 2
    tt = TRIANGLE_TABLE[ti]
    f1 = np.take_along_axis(idx_map[m1], tt[m1][:, :3].astype(np.int64), axis=1)
    f2 = np.take_along_axis(idx_map[m2], tt[m2][:, :6].astype(np.int64), axis=1).reshape(-1, 3)
    faces = np.concatenate([f1, f2], 0).astype(np.int32)
    nt = indices.shape[0]
    tg = np.arange(nt)[valid]
    fg = np.concatenate([tg[m1] * 2, np.stack([tg[m2] * 2, tg[m2] * 2 + 1], -1).reshape(-1)], 0)
    N = int(np.ceil(np.sqrt((nt * 2 + 1) // 2)))
    lin = np.linspace(0.0, 1.0 - 1.0 / N, N, dtype=np.float32)
    ty, tx = np.meshgrid(lin, lin, indexing='ij')
    pad = np.float32(0.9 / N)
    uvs = np.stack([tx, ty, tx + pad, ty, tx + pad, ty + pad, tx, ty + pad], -1).reshape(-1, 2).astype(np.float32)
    tix = (fg // 2 // N) * N + (fg // 2 % N)
    tri = fg % 2
    uv_idx = np.stack([tix * 4, tix * 4 + tri + 1, tix * 4 + tri + 2], -1).astype(np.int32)
    return ov, faces, uvs, uv_idx


def kernel(verts, sdf, deform, indices, grid_res):
    from concourse.bass_utils import run_bass_kernel_spmd
    verts = np.asarray(verts, dtype=np.float32)
    sdf = np.asarray(sdf, dtype=np.float32)
    deform = np.asarray(deform, dtype=np.float32)
    indices = np.asarray(indices)
    gr = int(np.asarray(grid_res))
    if gr != R or indices.shape != (R * R * R * 6, 4) or             not np.array_equal(indices.astype(np.int64), _expected_tets()):
        return _general_fallback(verts, sdf, deform, indices.astype(np.int64), gr)

    vd = (verts + (2.0 / (gr * 2)) * np.tanh(deform, dtype=np.float32)).astype(np.float32)
    sdf3 = sdf.reshape(NV, NV, NV)
    vd3 = vd.reshape(NV, NV, NV, 3)

    nc, _ = _build_bass()
    in_maps = _host_prep(sdf3, vd3)
    res = run_bass_kernel_spmd(nc, in_maps, list(range(8)))
    _LAST_RESULTS['res'] = res
    return _host_post(res.results, sdf3)
